# revision 49
# baseline (speedup 1.0000x reference)
"""Trainium2 Bass kernel for nn_Post_Prob (segment_reduce).

Reference computation, per image b (N=512 points, M=64*64=4096 pixels):
    dis[p, ij]  = (y_p - cood_i)^2 + (x_p - cood_j)^2
    min_dis[ij] = relu(min_p dis[p, ij])
    bg[ij]      = (0.15 * st_b)^2 / (min_dis + 1e-5)
    out         = softmax over the 513 rows of [-dis/128 ; -bg/128]

Sharding: data-parallel over the batch axis, 16 images -> 8 cores x 2.

Slot-window design: points are y-sorted on the host, so point index
("slot") tracks y within ~27 px.  Every per-pixel-chunk computation
then touches only a per-chunk contiguous slot window, fitted to the
actual data at build time (canonical stride-16 layout when it covers;
numpy fallback for pathological layouts):
  - matmul window: 256 slots centred on the chunk's y (padded pts array
    keeps it unclamped; fp32r needs >=256 output columns for 1 cyc/row)
  - work window W0=112 slots: exp (bf16, quad-batched via strided PSUM
    APs) / per-chunk 4x-mode tensor_scalar sum-accums / tt-max tree
  - min_dis = -128*ln(max of exp) -- turns the PSUM min-reduce into
    cheap 2x-mode SBUF tt-maxes and two small fin ops
  - output: ea is scaled in place by 1/Z (4x tensor_scalar), PE-
    transposed (bf16, 1 cyc/row) to [slot, px] PSUM, converted to f32
    by ACT copies (GPSIMD cannot read PSUM), and DMA'd with batched 3D
    APs (rows advance 16/chunk, 512B-contiguous rows).
Per-core pipeline: 2-stage software pipeline over 4 half-images --
A(s) [mm+exp+sums+max+fin] runs one step ahead of B(s) [scale+
transpose+copy+DMA]; copies/DMAs are emitted at step end so exps are
never head-of-line blocked on ACT.
Values outside the windows are provably below ~1.6e-3 of the softmax
scale (the background term keeps Z >= 0.55); bf16 adds ~0.4% relative
noise; measured Frobenius rel err is ~1.8e-3 (CPU-flavor inputs) and
~4e-4 (neuron-flavor), well inside the 2e-2 gate.
"""

import numpy as np

SIGMA = 8.0
C_SIZE = 512
STRIDE = 8
BG_RATIO = 0.15
EPS = 1e-5
B, N = 16, 512
C = C_SIZE // STRIDE  # 64
M = C * C  # 4096
NCORES = 8
BLOC = B // NCORES  # 2 images per core
INV = -1.0 / (2.0 * SIGMA * SIGMA)  # -1/128
CENTER = 256.0
NCHUNK = M // 128  # 32 pixel chunks (2 grid rows each)
PAD = 128  # slot padding each side of the 512 real slots
NPTS = N + 2 * PAD  # 768
DUMMY_Y = 8000.0
W0 = 112  # write/work window (slots)
T_WRITE = 30.0  # |dy| that must be inside the write window
T_SUM = 30.0  # |dy| that must be inside the work window

_CACHE = {}


def _split(v, bits=11):
    """v = hi + lo with hi keeping `bits` explicit mantissa bits."""
    u = np.ascontiguousarray(v, dtype=np.float32).view(np.uint32)
    hi = (u & np.uint32((0xFFFFFFFF << (23 - bits)) & 0xFFFFFFFF)).view(np.float32)
    lo = (v - hi).astype(np.float32)
    return hi, lo


def _host_consts():
    import ml_dtypes

    cood = (np.arange(0, C_SIZE, STRIDE, dtype=np.float32) + STRIDE / 2.0).astype(
        np.float32
    )
    cc = cood - np.float32(CENTER)
    ci = np.repeat(cc, C).astype(np.float32)  # i (y) varies slow over ij
    cj = np.tile(cc, C).astype(np.float32)  # j (x) varies fast
    c2 = (ci * ci + cj * cj).astype(np.float32)
    ones = np.ones(M, np.float32)
    zero = np.zeros(M, np.float32)
    ahi, alo = _split(-2.0 * ci)
    bhi, blo = _split(-2.0 * cj)
    chi, clo = _split(c2)
    const16 = np.stack(
        [
            ahi, alo, ahi, alo,       # k=0: * (y'hi, y'hi, y'lo, y'lo)
            bhi, blo, bhi, blo,       # k=1: * (x'hi, x'hi, x'lo, x'lo)
            ones, zero, ones, zero,   # k=2: * (r2hi, r2hi, r2lo, r2lo)
            chi, clo, zero, zero,     # k=3: * (1, 1, 1, 1)
        ]
    ).astype(np.float32)  # [16, M]
    ident = np.eye(128, dtype=np.float32)
    identb = np.eye(128).astype(ml_dtypes.bfloat16)
    return const16, ident, identb


def _windows(ow=None):
    """Per-chunk static slot windows (padded coords for matmul/work)."""
    o_mm = [16 * k + 8 for k in range(NCHUNK)]  # padded start of 256-window
    if ow is None:
        ow = [min(max(16 * k + 8 - W0 // 2, 0), N - W0) for k in range(NCHUNK)]
    w_off = [ow[k] + PAD - o_mm[k] for k in range(NCHUNK)]  # within [0,256-W0]
    return o_mm, list(ow), w_off


def _fit_windows(points):
    """Choose per-chunk window starts covering this data's candidate
    slots.  Returns (ow tuple, grouping) or None if infeasible.  Prefers
    the canonical stride-16 layout; falls back to per-chunk shifts with
    degraded DMA batching."""
    canon = [min(max(16 * k + 8 - W0 // 2, 0), N - W0) for k in range(NCHUNK)]
    lo = [0] * NCHUNK
    hi = [-1] * NCHUNK
    for bb in range(points.shape[0]):
        y = np.sort(points[bb, :, 1])
        for k in range(NCHUNK):
            c = 16 * k + 8
            m = np.abs(y - c) <= T_WRITE
            if m.any():
                idx = np.nonzero(m)[0]
                lo[k] = min(lo[k], int(idx[0])) if hi[k] >= 0 else int(idx[0])
                hi[k] = max(hi[k], int(idx[-1]))
    ow = []
    for k in range(NCHUNK):
        if hi[k] < 0:
            ow.append(canon[k])
            continue
        if hi[k] - lo[k] >= W0:
            return None  # window cannot cover; numpy fallback
        o = min(max(canon[k], hi[k] - W0 + 1), lo[k])
        o = min(max(o, 0), N - W0)
        if o > lo[k] or o + W0 <= hi[k]:
            return None
        # matmul 256-window must contain the work window
        if not (0 <= o + PAD - (16 * k + 8) <= 256 - W0):
            return None
        ow.append(o)
    return tuple(ow)


def _dma_groups(ow):
    """Split each half's 16 chunks into maximal uniform-stride runs."""
    groups = {}
    for h in (0, 1):
        k0, k1 = 16 * h, 16 * h + 16
        runs = []
        s = k0
        while s < k1:
            cap = k0 + 8 if s < k0 + 8 else k1  # break at copy boundary
            e = s + 1
            if e < cap:
                d = ow[e] - ow[e - 1]
                while e < cap and ow[e] - ow[e - 1] == d:
                    e += 1
            runs.append((s, e - s))
            s = e
        groups[h] = runs
    return groups


def _quad_spans(w_off):
    """Per quad: (base, width) covering its 4 chunks' work windows."""
    spans = []
    for q in range(NCHUNK // 4):
        offs = w_off[4 * q : 4 * q + 4]
        lo, hi = min(offs), max(offs) + W0
        spans.append((lo, hi - lo))
    return spans


def _force_combined_act_table(arch="gen3"):
    """Keep exp+ln+copy in one activation table (avoids ~2.7us reloads)."""
    import concourse.hw_specs as hw_specs

    tabs = hw_specs.get_activation_tables(arch)
    keep = "natural_log_exp_and_others"
    if keep in tabs:
        for name, s in tabs.items():
            if name != keep:
                s.clear()


def _build(copy_eng=("act", "act"), ow_t=None, pst_bufs=2, warm_n=0, ea_bufs=12, outb_bufs=3, psa_bufs=3):
    import concourse.bacc as bacc
    import concourse.tile as tile
    import concourse.mybir as mybir
    from concourse.ap import AP

    _force_combined_act_table()

    f32 = mybir.dt.float32
    f32r = mybir.dt.float32r
    bf16 = mybir.dt.bfloat16
    AF = mybir.ActivationFunctionType
    OP = mybir.AluOpType
    AX = mybir.AxisListType

    o_mm, ow, w_off = _windows(ow_t)
    spans = _quad_spans(w_off)
    dgroups = _dma_groups(ow)
    NQ = NCHUNK // 4  # 8 quads per image

    def _ap(base_ap, extra_off, dims):
        """Custom-stride sub-AP of an existing AP (element units)."""
        pdim = [int(base_ap.ap[0][0]), int(base_ap.ap[0][1])]
        return AP(base_ap.tensor, base_ap.offset + extra_off, [pdim] + dims)

    nc = bacc.Bacc("TRN2", target_bir_lowering=False, debug=False, num_devices=NCORES)

    pts_d = nc.dram_tensor("pts", [BLOC, 16, NPTS], f32r, kind="ExternalInput")
    const16_d = nc.dram_tensor("const16", [16, M], f32r, kind="ExternalInput")
    sbg_d = nc.dram_tensor("sbg", [128, BLOC], f32, kind="ExternalInput")
    id_d = nc.dram_tensor("ident", [128, 128], f32, kind="ExternalInput")
    idb_d = nc.dram_tensor("identb", [128, 128], bf16, kind="ExternalInput")
    out_d = nc.dram_tensor("out", [BLOC, N + 1, M], f32, kind="ExternalOutput")

    EWM = max(w for _, w in spans)  # 176 on this layout

    with tile.TileContext(nc) as tc:
        with (
            tc.tile_pool(name="singles", bufs=1) as singles,
            tc.tile_pool(name="psA", bufs=psa_bufs, space="PSUM") as psA_pool,
            tc.tile_pool(name="pst", bufs=pst_bufs, space="PSUM") as pst_pool,
            tc.tile_pool(name="warmps", bufs=1, space="PSUM") as warm_pool,
            tc.tile_pool(name="ea", bufs=ea_bufs) as ea_pool,
            tc.tile_pool(name="outb", bufs=outb_bufs) as outb_pool,
            tc.tile_pool(name="cols", bufs=1) as cols_pool,
            tc.tile_pool(name="fin", bufs=2) as fin_pool,
        ):
            # input loads: first half of image 0 is on the critical path
            const16_t = singles.tile([16, M], f32r)
            pts_tiles = []
            for bb in range(BLOC):
                pt = singles.tile([16, NPTS], f32r, tag=f"pts{bb}")
                pts_tiles.append(pt)
            # first A-quad needs pts0 + const16 cols 0:512 only; ACT ring
            # carries just pts0 so the first exp isn't queued behind bulk,
            # Pool SWDGE takes the small constants (Pool idles early on)
            nc.sync.dma_start(const16_t[:, 0:512], const16_d[:, 0:512])
            nc.gpsimd.dma_start(pts_tiles[0][:], pts_d[0])
            idb_t = singles.tile([128, 128], bf16)
            nc.scalar.dma_start(idb_t[:], idb_d[:])
            nc.sync.dma_start(const16_t[:, 512:2048], const16_d[:, 512:2048])
            id_t = singles.tile([128, 128], f32)
            nc.gpsimd.dma_start(id_t[:], id_d[:])
            sbg_t = singles.tile([128, BLOC], f32)
            nc.gpsimd.dma_start(sbg_t[:], sbg_d[:])
            nc.scalar.dma_start(pts_tiles[1][:], pts_d[1])
            nc.sync.dma_start(const16_t[:, 2048:], const16_d[:, 2048:])

            cols = {}
            for bb in range(BLOC):
                cols[bb] = {}
                for nm, dt_ in (("mx", bf16), ("sm", f32), ("rz", f32),
                                ("bgp", f32)):
                    cols[bb][nm] = cols_pool.tile(
                        [128, NCHUNK], dt_, tag=f"{nm}{bb}", name=f"{nm}{bb}"
                    )

            eas = {}  # (bb, h) -> list of 4 ea tiles
            psts = {}  # (bb, h, g) -> pst tile

            def emit_A_mm(bb, h, qh):
                q = 4 * h + qh
                ps = psA_pool.tile([128, 4, 256], f32, tag="psA")
                for j in range(4):
                    k = 4 * q + j
                    nc.tensor.matmul(
                        ps[:, j, :],
                        const16_t[:, k * 128 : (k + 1) * 128],
                        pts_tiles[bb][:, o_mm[k] : o_mm[k] + 256],
                        start=True,
                        stop=True,
                    )
                return ps

            def emit_A_exp(bb, h, qh, ps):
                q = 4 * h + qh
                offs = w_off[4 * q : 4 * q + 4]
                dq = offs[1] - offs[0]  # 0 interior, -16 at the edges
                ea = ea_pool.tile([128, 4, W0], bf16, tag="ea", name="ea")
                if all(offs[j] == offs[0] + dq * j for j in range(4)):
                    src = _ap(ps[:], offs[0], [[256 + dq, 4], [1, W0]])
                    nc.scalar.activation(
                        ea[:], src, AF.Exp, bias=0.0, scale=INV
                    )
                else:  # fitted windows: per-chunk offsets, unbatched
                    for j in range(4):
                        nc.scalar.activation(
                            ea[:, j, :], ps[:, j, offs[j] : offs[j] + W0],
                            AF.Exp, bias=0.0, scale=INV,
                        )
                eas[(bb, h)][qh] = ea
                return ea

            def emit_A_sums(bb, h, qh, ea):
                """Per-chunk sum-of-exp via 4x-mode tensor_scalar accum."""
                q = 4 * h + qh
                sm = cols[bb]["sm"]
                for j in range(4):
                    k = 4 * q + j
                    nc.vector.tensor_scalar(
                        ea[:, j, :], ea[:, j, :], 1.0, 0.0,
                        op0=OP.mult, op1=OP.add, accum_out=sm[:, k : k + 1],
                    )

            def emit_A_max(bb, h):
                """Half-level window max: two 2x tt-max halvings + reduce."""
                cc = cols[bb]
                qs = eas[(bb, h)]
                hw_, qw = W0 // 2, W0 // 4
                t1 = fin_pool.tile([128, 4, 4, hw_], bf16, tag="t1", name="t1")
                for qh in range(4):
                    ea = qs[qh]
                    nc.vector.tensor_tensor(
                        t1[:, qh, :, :], ea[:, :, 0:hw_], ea[:, :, hw_ : 2 * hw_],
                        op=OP.max,
                    )
                nc.vector.tensor_tensor(
                    t1[:, :, :, 0:qw], t1[:, :, :, 0:qw],
                    t1[:, :, :, qw : 2 * qw], op=OP.max,
                )
                nc.vector.tensor_reduce(
                    cc["mx"][:, 16 * h : 16 * h + 16],
                    t1[:, :, :, 0:qw], axis=AX.X, op=OP.max,
                )

            def emit_scale_quad(bb, h, qh):
                q = 4 * h + qh
                ea = eas[(bb, h)][qh]
                rz = cols[bb]["rz"]
                for j in range(4):
                    k = 4 * q + j
                    nc.vector.tensor_scalar_mul(
                        ea[:, j, :], ea[:, j, :], rz[:, k : k + 1]
                    )

            def emit_T_quad(bb, h, qh):
                ea = eas[(bb, h)][qh]
                g = qh // 2
                if qh % 2 == 0:
                    psts[(bb, h, g)] = pst_pool.tile([128, 8, 128], bf16, tag="pst", name="pst")
                ps = psts[(bb, h, g)]
                for j in range(4):
                    nc.tensor.transpose(
                        ps[:W0, 4 * (qh % 2) + j, :], ea[:, j, :], idb_t[:]
                    )

            def emit_copy(bb, h, g, eng):
                """copy a transposed 8-chunk group psum->sbuf (+f32 convert).
                GPSIMD cannot read PSUM, so only ACT/DVE are legal here."""
                outsb = outsbs[(bb, h)]
                ps = psts[(bb, h, g)]
                dst = outsb[:W0, 8 * g : 8 * g + 8, :]
                srcg = ps[:W0, :, :]
                if eng == "act":
                    nc.scalar.copy(dst, srcg)
                else:
                    nc.vector.tensor_copy(dst, srcg)

            def emit_dma(bb, h, gk0, gn):
                """output rows for gn chunks: [W0 slots, gn chunks, 128 px]."""
                outsb = outsbs[(bb, h)]
                row0 = ow[gk0]
                drow = ow[gk0 + 1] - ow[gk0] if gn > 1 else 0
                bs = out_d[bb]
                dst = AP(
                    bs.tensor,
                    bs.offset + row0 * M + gk0 * 128,
                    [[M, W0], [drow * M + 128, gn], [1, 128]],
                )
                c0 = gk0 - 16 * h
                nc.sync.dma_start(dst, outsb[:W0, c0 : c0 + gn, :])

            def emit_fin_half(bb, h):
                """chunks 16h..16h+15: bg row segment + 1/Z per pixel."""
                cc = cols[bb]
                s = slice(16 * h, 16 * (h + 1))
                lnm = fin_pool.tile([128, 16], f32, tag="lnm")
                nc.scalar.activation(lnm[:], cc["mx"][:, s], AF.Ln, bias=0.0, scale=1.0)
                # min = -128 * clamp(ln(max), -90, 0); then *(-128) + eps
                nc.vector.tensor_scalar(
                    lnm[:], lnm[:], -90.0, 0.0, op0=OP.max, op1=OP.min
                )
                tmp = fin_pool.tile([128, 16], f32, tag="tmp")
                nc.vector.tensor_scalar(
                    tmp[:], lnm[:], -(2.0 * SIGMA * SIGMA), EPS,
                    op0=OP.mult, op1=OP.add,
                )
                rmin = fin_pool.tile([128, 16], f32, tag="rmin")
                nc.vector.reciprocal(rmin[:], tmp[:])
                bgd = fin_pool.tile([128, 16], f32, tag="bgd")
                nc.vector.tensor_scalar_mul(bgd[:], rmin[:], sbg_t[:, bb : bb + 1])
                ebg = fin_pool.tile([128, 16], f32, tag="ebg")
                nc.scalar.activation(ebg[:], bgd[:], AF.Exp, bias=0.0, scale=INV)
                stot = fin_pool.tile([128, 16], f32, tag="stot")
                nc.vector.tensor_tensor(stot[:], cc["sm"][:, s], ebg[:], op=OP.add)
                nc.vector.reciprocal(cc["rz"][:, s], stot[:])
                nc.vector.tensor_tensor(
                    cc["bgp"][:, s], ebg[:], cc["rz"][:, s], op=OP.mult
                )

            def emit_bg(bb):
                """background row: transpose bg probs, copy, one 16KB DMA."""
                ps = pst_pool.tile([128, 8, 128], bf16, tag="pst", name="pst")
                psf = ps[:32, 0:2, :].bitcast(f32)
                nc.tensor.transpose(psf, cols[bb]["bgp"][:], id_t[:])
                tsb = fin_pool.tile([32, 128], f32, tag="tsb")
                nc.scalar.copy(tsb[:], psf)
                nc.sync.dma_start(
                    out_d[bb, N, :].rearrange("(k q) -> k q", q=128), tsb[:]
                )

            # warm the activation table at t=0, and ramp the PE p-state
            # with a ~3us dummy matmul chain while the inputs stream in
            warm = fin_pool.tile([1, 1], f32, tag="warm")
            nc.vector.memset(warm[:], 0.0)
            nc.scalar.activation(warm[:], warm[:], AF.Exp, bias=0.0, scale=1.0)
            if warm_n:
                wsrc = singles.tile([16, 128], f32)
                nc.vector.memset(wsrc[:], 0.0)
                wps = warm_pool.tile([128, 128], f32)
                for _ in range(warm_n):
                    nc.tensor.matmul(wps[:], wsrc[:], wsrc[:, 0:128],
                                     start=True, stop=True)

            seq = [(bb, h) for bb in range(BLOC) for h in range(2)]
            outsbs = {}

            def emit_B_quad(bb, h, qh):
                emit_T_quad(bb, h, qh)

            def emit_B_drain(bb, h, last=False):
                """copies + DMAs; emitted late so ACT exps aren't blocked.
                For the final half, drain per quad so the serial DMA burst
                starts as early as possible."""
                hi = 2 * (2 * bb + h)
                runs = dgroups[h]
                if last:
                    for qh in range(4):
                        g, j = divmod(qh, 2)
                        ps = psts[(bb, h, g)]
                        outsb = outsbs[(bb, h)]
                        nc.scalar.copy(
                            outsb[:W0, 4 * qh : 4 * qh + 4, :],
                            ps[:W0, 4 * j : 4 * j + 4, :],
                        )
                        for gk0, gn in runs:
                            if 16 * h + 4 * qh <= gk0 and gk0 + gn <= 16 * h + 4 * qh + 4:
                                emit_dma(bb, h, gk0, gn)
                    return
                emit_copy(bb, h, 0, copy_eng[hi % len(copy_eng)])
                for gk0, gn in runs:
                    if gk0 + gn <= 16 * h + 8:
                        emit_dma(bb, h, gk0, gn)
                emit_copy(bb, h, 1, copy_eng[(hi + 1) % len(copy_eng)])
                for gk0, gn in runs:
                    if gk0 + gn > 16 * h + 8:
                        emit_dma(bb, h, gk0, gn)

            prev = None
            for i, (bb, h) in enumerate(seq):
                eas[(bb, h)] = [None] * 4
                if prev is not None:
                    outsbs[prev] = outb_pool.tile(
                        [128, 16, 128], f32, tag="outsb", name="outsb"
                    )
                    for qh in range(4):
                        emit_scale_quad(*prev, qh)
                # PE: two matmul quads ahead of the B transposes
                pss = []
                for qh in range(6):
                    if qh < 4:
                        pss.append(emit_A_mm(bb, h, qh))
                        emit_A_exp(bb, h, qh, pss[qh])
                    if qh >= 2 and prev is not None:
                        emit_B_quad(*prev, qh - 2)
                for qh in range(4):
                    emit_A_sums(bb, h, qh, eas[(bb, h)][qh])
                emit_A_max(bb, h)
                emit_fin_half(bb, h)
                if h == 1:
                    emit_bg(bb)
                if prev is not None:
                    emit_B_drain(*prev)
                    for g in (0, 1):
                        psts.pop((prev[0], prev[1], g), None)
                    eas.pop(prev)
                    outsbs.pop(prev)
                prev = (bb, h)

            # epilogue: B for the final half
            outsbs[prev] = outb_pool.tile(
                [128, 16, 128], f32, tag="outsb", name="outsb"
            )
            for qh in range(4):
                emit_scale_quad(*prev, qh)
            for qh in range(4):
                emit_B_quad(*prev, qh)
            emit_B_drain(*prev)

    nc.compile()
    return nc


def _get_nc(cfg=()):
    key = ("nc",) + tuple(cfg)
    if key not in _CACHE:
        _CACHE[key] = _build(*cfg) if cfg else _build()
        try:
            from concourse.timeline_sim import TimelineSim

            _CACHE[("nc", "est")] = TimelineSim(_CACHE[key], trace=False).simulate()
        except Exception:
            pass
    if "consts" not in _CACHE:
        _CACHE["consts"] = _host_consts()
    return _CACHE[key]


def _in_maps(points, st_sizes):
    points = np.ascontiguousarray(np.asarray(points, dtype=np.float32))
    st_sizes = np.asarray(st_sizes, dtype=np.float32)
    const16, ident, identb = _CACHE["consts"]
    in_maps = []
    for c in range(NCORES):
        sl = slice(BLOC * c, BLOC * (c + 1))
        p = points[sl]  # [BLOC, N, 2]
        p = np.stack(
            [p[bb][np.argsort(p[bb, :, 1], kind="stable")] for bb in range(BLOC)]
        )
        # pad with far-away dummies so slot windows never clamp
        pp = np.empty((BLOC, NPTS, 2), np.float32)
        pp[:, :, 0] = 0.0
        pp[:, :, 1] = DUMMY_Y
        pp[:, PAD : PAD + N] = p
        xk = (pp[..., 0] - np.float32(CENTER)).astype(np.float32)
        yk = (pp[..., 1] - np.float32(CENTER)).astype(np.float32)
        r2 = (xk * xk + yk * yk).astype(np.float32)
        yhi, ylo = _split(yk)
        xhi, xlo = _split(xk)
        rhi, rlo = _split(r2)
        one = np.ones_like(xk)
        pts = np.ascontiguousarray(
            np.stack(
                [yhi, yhi, ylo, ylo, xhi, xhi, xlo, xlo, rhi, rhi, rlo, rlo,
                 one, one, one, one],
                axis=1,
            )
        )  # [BLOC, 16, NPTS]
        s = ((st_sizes[sl] * np.float32(BG_RATIO)) ** 2).astype(np.float32)
        sbg = np.ascontiguousarray(np.broadcast_to(s[None, :], (128, BLOC)))
        in_maps.append(
            {"pts": pts, "const16": const16, "sbg": sbg, "ident": ident,
             "identb": identb}
        )
    return in_maps


def _numpy_fallback(points, st_sizes):
    """Dense host computation for pathological point layouts."""
    cood = np.arange(0, C_SIZE, STRIDE, dtype=np.float64) + STRIDE / 2.0
    out = np.empty((B, N + 1, M), np.float32)
    for bb in range(B):
        x = points[bb, :, 0].astype(np.float64)
        y = points[bb, :, 1].astype(np.float64)
        xd = (x[:, None] - cood) ** 2
        yd = (y[:, None] - cood) ** 2
        dis = (yd[:, :, None] + xd[:, None, :]).reshape(N, M)
        mind = np.clip(dis.min(axis=0), 0.0, None)
        bg = (float(st_sizes[bb]) * BG_RATIO) ** 2 / (mind + EPS)
        logits = np.concatenate([dis, bg[None]], axis=0) * (-1.0 / 128.0)
        e = np.exp(logits - logits.max(axis=0))
        out[bb] = (e / e.sum(axis=0)).astype(np.float32)
    return out


def _run(points, st_sizes, trace=False):
    from concourse.bass_utils import run_bass_kernel_spmd

    points = np.ascontiguousarray(np.asarray(points, dtype=np.float32))
    ow_t = _fit_windows(points)
    if ow_t is None:
        return _numpy_fallback(points, np.asarray(st_sizes)), None
    canon = tuple(_windows()[1])
    nc = _get_nc((("act", "act"), None if ow_t == canon else ow_t))
    _CACHE["last_est"] = _CACHE.get(("nc", "est"))
    res = run_bass_kernel_spmd(
        nc, _in_maps(points, st_sizes), core_ids=list(range(NCORES)),
        trace=trace,
    )
    out = np.concatenate([r["out"] for r in res.results], axis=0)
    # rows are in y-sorted order on device; scatter back to input order
    perm = np.argsort(points[..., 1], axis=1, kind="stable")  # [B, N]
    full = np.zeros_like(out)
    for bb in range(B):
        full[bb, perm[bb], :] = out[bb, :N, :]
        full[bb, N, :] = out[bb, N, :]
    return full, res


def kernel(points, st_sizes):
    out, _ = _run(points, st_sizes, trace=False)
    return out


def kernel_profiled(points, st_sizes):
    """Returns (out, BassKernelResults) with exec_time_ns populated."""
    return _run(points, st_sizes, trace=True)
